# revision 1
# baseline (speedup 1.0000x reference)
"""Two-layer GAT on 8 Trainium2 NeuronCores — old gather structure + bf16.

Same design as the original baseline (dense dst-window edge columns, int32
indirect gathers, one-hot scatter with PE-transpose a_dst expansion) but with
bf16 tables, gathers, one-hots, and matmuls: ~2x less gather traffic and ~4x
faster PE ops. Scores/softmax stay fp32. Epsilon added to softmax denominators
(reference does the same) so empty pad rows yield finite garbage instead of
NaN that poisons layer-2 matmuls.
"""

import numpy as np

# ---- problem constants (hardcoded per harness contract) ----
N = 100000
E = 1600000
IN = 128
HID = 16
HEADS = 8
OUT = 64
NEG = 0.2
NC = 8
NLOC = N // NC          # 12500
WIN = 128
NWIN = (NLOC + WIN - 1) // WIN   # 98
LAST_ROWS = NLOC - (NWIN - 1) * WIN  # 84
C1 = HEADS * HID        # 128
ROW1 = C1 + 2 * HEADS   # 144 = z(128) | a_src(8) | a_dst(8)
ROW2 = OUT + 2          # 66  = h2(64) | a_src2(1) | a_dst2(1)
BATCH = 8               # edge groups per batched compute slab
PHA_B = 3               # Phase A node tiles per psum bank (3*144=432 <= 512)


def _preprocess(edge_index):
    """Per-core edge arrays in partition-major layout + shared group counts."""
    src = np.concatenate([np.asarray(edge_index[0]), np.arange(N)]).astype(np.int64)
    dst = np.concatenate([np.asarray(edge_index[1]), np.arange(N)]).astype(np.int64)
    core = dst // NLOC
    per_core = []
    cnts = np.zeros((NC, NWIN), dtype=np.int64)
    for k in range(NC):
        m = core == k
        s, d = src[m], dst[m] - k * NLOC
        o = np.argsort(d, kind="stable")
        s, d = s[o], d[o]
        per_core.append((s, d))
        cnts[k] = np.bincount(d // WIN, minlength=NWIN)
    ngroups = np.maximum(1, ((cnts + 127) // 128).max(axis=0))  # shared, >=1
    G = int(ngroups.sum())
    gstart = np.concatenate([[0], np.cumsum(ngroups)])
    srcsT = np.zeros((NC, 128, G), dtype=np.int32)
    dstwT = np.full((NC, 128, G), 999.0, dtype=np.float32)
    for k in range(NC):
        s, d = per_core[k]
        w = d // WIN
        ws = np.searchsorted(w, np.arange(NWIN))
        we = np.searchsorted(w, np.arange(NWIN), side="right")
        fs = np.zeros(G * 128, dtype=np.int64)
        fw = np.full(G * 128, 999.0, dtype=np.float32)
        for wi in range(NWIN):
            cnt = we[wi] - ws[wi]
            a = gstart[wi] * 128
            fs[a:a + cnt] = s[ws[wi]:we[wi]]
            fw[a:a + cnt] = (d[ws[wi]:we[wi]] - wi * WIN).astype(np.float32)
        srcsT[k] = fs.reshape(G, 128).T.astype(np.int32)
        dstwT[k] = fw.reshape(G, 128).T
    return srcsT, dstwT, ngroups.tolist(), G


def _pack_weights(W1, att_src1, att_dst1, W2, att_src2, att_dst2):
    import ml_dtypes
    W1 = np.asarray(W1, np.float32)
    W2 = np.asarray(W2, np.float32)
    A1s = np.zeros((C1, HEADS), np.float32)
    A1d = np.zeros((C1, HEADS), np.float32)
    for h in range(HEADS):
        A1s[h * HID:(h + 1) * HID, h] = np.asarray(att_src1, np.float32)[h]
        A1d[h * HID:(h + 1) * HID, h] = np.asarray(att_dst1, np.float32)[h]
    W1ext = np.concatenate([W1, W1 @ A1s, W1 @ A1d], axis=1)   # [128, 144]
    W2ext = np.concatenate(
        [W2, W2 @ np.asarray(att_src2, np.float32).T,
         W2 @ np.asarray(att_dst2, np.float32).T], axis=1)     # [128, 66]
    return (np.ascontiguousarray(W1ext).astype(ml_dtypes.bfloat16),
            np.ascontiguousarray(W2ext).astype(ml_dtypes.bfloat16))


def _build_nc(ngroups, G):
    import concourse.bass as bass
    import concourse.bacc as bacc
    import concourse.mybir as mybir
    import concourse.tile as tile

    dt = mybir.dt
    AF = mybir.ActivationFunctionType
    OP = mybir.AluOpType
    nc = bacc.Bacc("TRN2", target_bir_lowering=False, debug=False, num_devices=NC)

    xT = nc.dram_tensor("xT", [IN, N], dt.bfloat16, kind="ExternalInput")
    W1e = nc.dram_tensor("W1e", [IN, ROW1], dt.bfloat16, kind="ExternalInput")
    W2e = nc.dram_tensor("W2e", [C1, ROW2], dt.bfloat16, kind="ExternalInput")
    iota2d = nc.dram_tensor("iota2d", [128, 128], dt.bfloat16, kind="ExternalInput")
    ident = nc.dram_tensor("ident", [128, 128], dt.bfloat16, kind="ExternalInput")
    b1_2d = nc.dram_tensor("b1_2d", [128, C1], dt.float32, kind="ExternalInput")
    b2_2d = nc.dram_tensor("b2_2d", [128, OUT], dt.float32, kind="ExternalInput")
    srcsT = nc.dram_tensor("srcsT", [128, G], dt.int32, kind="ExternalInput")
    dstwT = nc.dram_tensor("dstwT", [128, G], dt.float32, kind="ExternalInput")
    xownT = nc.dram_tensor("xownT", [IN, NWIN * 128], dt.bfloat16, kind="ExternalInput")
    out = nc.dram_tensor("out", [NLOC, OUT], dt.float32, kind="ExternalOutput")

    hext1 = nc.dram_tensor("hext1", [N, ROW1], dt.bfloat16)
    adstloc = nc.dram_tensor("adstloc", [NWIN * 128, HEADS], dt.bfloat16)
    h2own = nc.dram_tensor("h2own", [NLOC, ROW2], dt.bfloat16)
    h2full = nc.dram_tensor("h2full", [N, ROW2], dt.bfloat16, addr_space="Shared")

    gstart = np.concatenate([[0], np.cumsum(ngroups)]).astype(int)

    with tile.TileContext(nc) as tc:
        with tc.tile_pool(name="const", bufs=1) as cb:
            w1e_t = cb.tile([IN, ROW1], dt.bfloat16)
            nc.sync.dma_start(out=w1e_t[:], in_=W1e[:, :])
            w2e_t = cb.tile([C1, ROW2], dt.bfloat16)
            nc.sync.dma_start(out=w2e_t[:], in_=W2e[:, :])
            iota_t = cb.tile([128, 128], dt.bfloat16)
            nc.sync.dma_start(out=iota_t[:], in_=iota2d[:, :])
            ident_t = cb.tile([128, 128], dt.bfloat16)
            nc.sync.dma_start(out=ident_t[:], in_=ident[:, :])
            b1_t = cb.tile([128, C1], dt.float32)
            nc.sync.dma_start(out=b1_t[:], in_=b1_2d[:, :])
            b2_t = cb.tile([128, OUT], dt.float32)
            nc.sync.dma_start(out=b2_t[:], in_=b2_2d[:, :])
            tc.strict_bb_all_engine_barrier()

            # ---------------- Phase A: hext1 = x @ W1ext (replicated) --------
            with (
                tc.tile_pool(name="pha_sb", bufs=3) as sa,
                tc.tile_pool(name="pha_ps", bufs=2, space="PSUM") as pa,
            ):
                ntile = (N + 127) // 128  # 782, last has 32 rows
                t = 0
                while t < ntile:
                    nb = min(PHA_B, ntile - t)
                    r0 = t * 128
                    rows = min(nb * 128, N - r0)
                    xt = sa.tile([IN, nb * 128], dt.bfloat16, tag="xt")
                    nc.sync.dma_start(out=xt[:, :rows], in_=xT[:, r0:r0 + rows])
                    psA = pa.tile([128, nb * ROW1], dt.float32, tag="psA")
                    for b in range(nb):
                        rr = min(128, N - (t + b) * 128)
                        nc.tensor.matmul(
                            out=psA[:rr, b * ROW1:(b + 1) * ROW1],
                            lhsT=xt[:, b * 128:b * 128 + rr],
                            rhs=w1e_t[:], start=True, stop=True)
                    zs = sa.tile([128, nb * ROW1], dt.bfloat16, tag="zs")
                    if rows % 128 == 0:
                        nc.vector.tensor_copy(out=zs[:], in_=psA[:])
                        nc.sync.dma_start(
                            out=hext1[r0:r0 + rows, :].rearrange(
                                "(b p) f -> p b f", p=128),
                            in_=zs[:].rearrange("p (b f) -> p b f", b=nb))
                    else:
                        for b in range(nb):
                            rr = min(128, N - (t + b) * 128)
                            nc.vector.tensor_copy(
                                out=zs[:rr, b * ROW1:(b + 1) * ROW1],
                                in_=psA[:rr, b * ROW1:(b + 1) * ROW1])
                            nc.sync.dma_start(
                                out=hext1[(t + b) * 128:(t + b) * 128 + rr, :],
                                in_=zs[:rr, b * ROW1:(b + 1) * ROW1])
                    t += nb
                # Phase A2: own-shard a_dst table (window-padded, core-local)
                for w in range(NWIN):
                    xo = sa.tile([IN, 128], dt.bfloat16, tag="xo")
                    nc.sync.dma_start(out=xo[:], in_=xownT[:, w * 128:(w + 1) * 128])
                    psA2 = pa.tile([128, HEADS], dt.float32, tag="psA2")
                    nc.tensor.matmul(out=psA2[:], lhsT=xo[:],
                                     rhs=w1e_t[:, C1 + HEADS:ROW1], start=True, stop=True)
                    a2s = sa.tile([128, HEADS], dt.bfloat16, tag="a2s")
                    nc.vector.tensor_copy(out=a2s[:], in_=psA2[:])
                    nc.sync.dma_start(out=adstloc[w * 128:(w + 1) * 128, :], in_=a2s[:])
            tc.strict_bb_all_engine_barrier()

            # ---------------- edge aggregation pipeline ----------------------
            def edge_layer(table_ap, feat, nh, adw_src, flush):
                S = feat + nh
                with (
                    tc.tile_pool(name="eb_sb", bufs=3) as sb,
                    tc.tile_pool(name="eb_idx", bufs=2) as sx,
                    tc.tile_pool(name="eb_ps", bufs=2, space="PSUM") as pw,
                    tc.tile_pool(name="eb_pot", bufs=2, space="PSUM") as pot,
                    tc.tile_pool(name="eb_pad", bufs=2, space="PSUM") as pad,
                    tc.tile_pool(name="eb_ps2", bufs=1, space="PSUM") as p2,
                ):
                    for w in range(NWIN):
                        g0, g1 = int(gstart[w]), int(gstart[w + 1])
                        ng = g1 - g0
                        src_t = sx.tile([128, ng], dt.int32, tag="src")
                        dw_t = sx.tile([128, ng], dt.float32, tag="dw")
                        nc.sync.dma_start(out=src_t[:], in_=srcsT[:, g0:g1])
                        nc.sync.dma_start(out=dw_t[:], in_=dstwT[:, g0:g1])
                        adw_ap, adw_rows = adw_src(w)
                        adw_t = sx.tile([128, nh], dt.bfloat16, tag="adw")
                        if adw_rows < 128:
                            nc.gpsimd.memset(adw_t[:], 0.0)
                        nc.sync.dma_start(out=adw_t[:adw_rows, :], in_=adw_ap)
                        psW = pw.tile([128, S], dt.float32, tag="psW")
                        j = 0
                        first = True
                        while j < ng:
                            nb = min(BATCH, ng - j)
                            hx = sb.tile([128, BATCH * S], dt.bfloat16, tag="hx")
                            ad = sb.tile([128, BATCH * nh], dt.float32, tag="ad")
                            for b in range(nb):
                                nc.gpsimd.indirect_dma_start(
                                    out=hx[:, b * S:(b + 1) * S],
                                    out_offset=None, in_=table_ap,
                                    in_offset=bass.IndirectOffsetOnAxis(
                                        ap=src_t[:, j + b:j + b + 1], axis=0))
                            Ot = sb.tile([128, BATCH * 128], dt.bfloat16, tag="Ot")
                            for b in range(nb):
                                nc.vector.tensor_scalar(
                                    out=Ot[:, b * 128:(b + 1) * 128], in0=iota_t[:],
                                    scalar1=dw_t[:, j + b:j + b + 1], scalar2=None,
                                    op0=OP.is_equal)
                            # a_dst_e = O @ a_dstW  (transpose O on PE, then matmul)
                            for b in range(nb):
                                psOT = pot.tile([128, 128], dt.bfloat16, tag="psOT")
                                nc.tensor.transpose(
                                    out=psOT[:], in_=Ot[:, b * 128:(b + 1) * 128],
                                    identity=ident_t[:])
                                ot_sb = sb.tile([128, 128], dt.bfloat16, tag="otsb")
                                nc.scalar.copy(out=ot_sb[:], in_=psOT[:])
                                psAD = pad.tile([128, nh], dt.float32, tag="psAD")
                                nc.tensor.matmul(out=psAD[:], lhsT=ot_sb[:],
                                                 rhs=adw_t[:], start=True, stop=True)
                                nc.scalar.copy(out=ad[:, b * nh:(b + 1) * nh], in_=psAD[:])
                            # e = a_src + a_dst ; w = exp(max(e, 0.2e))
                            ev = sb.tile([128, BATCH * nh], dt.float32, tag="ev")
                            asrc_v = hx[:].rearrange("p (b f) -> p b f", b=BATCH)[:, :nb, feat:S]
                            nc.vector.tensor_tensor(
                                out=ev[:, :nb * nh].rearrange("p (b h) -> p b h", b=nb),
                                in0=asrc_v, in1=ad[:, :nb * nh].rearrange(
                                    "p (b h) -> p b h", b=nb), op=OP.add)
                            sc = sb.tile([128, BATCH * nh], dt.float32, tag="sc")
                            nc.scalar.mul(out=sc[:, :nb * nh], in_=ev[:, :nb * nh], mul=NEG)
                            w8 = sb.tile([128, BATCH * nh], dt.float32, tag="w8")
                            nc.vector.tensor_tensor(out=w8[:, :nb * nh], in0=ev[:, :nb * nh],
                                                    in1=sc[:, :nb * nh], op=OP.max)
                            nc.scalar.activation(out=w8[:, :nb * nh], in_=w8[:, :nb * nh],
                                                 func=AF.Exp)
                            # weighted rhs
                            rhs = sb.tile([128, BATCH * S], dt.bfloat16, tag="rhs")
                            if nh > 1:
                                hx_v = hx[:].rearrange("p (b f) -> p b f", b=BATCH)[
                                    :, :nb, 0:feat].rearrange("p b (h c) -> p b h c", h=nh)
                                w8_v = w8[:, :nb * nh].rearrange(
                                    "p (b h) -> p b h", b=nb)[:, :, :, None].to_broadcast(
                                    [128, nb, nh, feat // nh])
                                rhs_v = rhs[:].rearrange("p (b f) -> p b f", b=BATCH)[
                                    :, :nb, 0:feat].rearrange("p b (h c) -> p b h c", h=nh)
                            else:
                                hx_v = hx[:].rearrange("p (b f) -> p b f", b=BATCH)[:, :nb, 0:feat]
                                w8_v = w8[:, :nb * nh].rearrange(
                                    "p (b h) -> p b h", b=nb).to_broadcast([128, nb, feat])
                                rhs_v = rhs[:].rearrange("p (b f) -> p b f", b=BATCH)[:, :nb, 0:feat]
                            nc.vector.tensor_tensor(out=rhs_v, in0=hx_v, in1=w8_v, op=OP.mult)
                            nc.vector.tensor_copy(
                                out=rhs[:].rearrange("p (b f) -> p b f", b=BATCH)[:, :nb, feat:S],
                                in_=w8[:, :nb * nh].rearrange("p (b h) -> p b h", b=nb))
                            for b in range(nb):
                                nc.tensor.matmul(
                                    out=psW[:], lhsT=Ot[:, b * 128:(b + 1) * 128],
                                    rhs=rhs[:, b * S:(b + 1) * S],
                                    start=first and b == 0,
                                    stop=(j + nb >= ng) and b == nb - 1)
                            first = False
                            j += nb
                        flush(w, psW, sb, p2)

            def flush1(w, psW, sb, p2):
                den = sb.tile([128, HEADS], dt.float32, tag="den")
                nc.vector.tensor_scalar(out=den[:], in0=psW[:, C1:C1 + HEADS],
                                        scalar1=1e-16, scalar2=None, op0=OP.add)
                recip = sb.tile([128, HEADS], dt.float32, tag="recip")
                nc.vector.reciprocal(out=recip[:], in_=den[:])
                A = sb.tile([128, C1], dt.bfloat16, tag="A")
                nc.vector.tensor_tensor(
                    out=A[:].rearrange("p (h c) -> p h c", h=HEADS),
                    in0=psW[:, 0:C1].rearrange("p (h c) -> p h c", h=HEADS),
                    in1=recip[:][:, :, None].to_broadcast([128, HEADS, HID]),
                    op=OP.mult)
                nc.vector.tensor_tensor(out=A[:], in0=A[:], in1=b1_t[:], op=OP.add)
                nc.scalar.activation(out=A[:], in_=A[:], func=AF.Relu)
                psT = p2.tile([128, 128], dt.bfloat16, tag="psT")
                nc.tensor.transpose(out=psT[:], in_=A[:], identity=ident_t[:])
                at = sb.tile([128, 128], dt.bfloat16, tag="at")
                nc.scalar.copy(out=at[:], in_=psT[:])
                ps2 = p2.tile([128, ROW2], dt.float32, tag="ps2")
                nc.tensor.matmul(out=ps2[:], lhsT=at[:], rhs=w2e_t[:], start=True, stop=True)
                h2sb = sb.tile([128, ROW2], dt.bfloat16, tag="h2sb")
                nc.vector.tensor_copy(out=h2sb[:], in_=ps2[:])
                rows = 128 if w < NWIN - 1 else LAST_ROWS
                nc.sync.dma_start(out=h2own[w * 128:w * 128 + rows, :], in_=h2sb[:rows, :])

            edge_layer(hext1[:, :], C1, HEADS,
                       lambda w: (adstloc[w * 128:(w + 1) * 128, :], 128), flush1)
            tc.strict_bb_all_engine_barrier()

            nc.gpsimd.collective_compute(
                "AllGather", OP.bypass,
                replica_groups=[list(range(NC))],
                ins=[h2own[:, :]], outs=[h2full[:, :]])
            tc.strict_bb_all_engine_barrier()

            # ---------------- Phase C: layer-2 edge aggregation --------------
            def flush2(w, psW, sb, p2):
                den = sb.tile([128, 1], dt.float32, tag="den2")
                nc.vector.tensor_scalar(out=den[:], in0=psW[:, OUT:OUT + 1],
                                        scalar1=1e-16, scalar2=None, op0=OP.add)
                recip = sb.tile([128, 1], dt.float32, tag="recip2")
                nc.vector.reciprocal(out=recip[:], in_=den[:])
                o2 = sb.tile([128, OUT], dt.float32, tag="o2")
                nc.vector.tensor_tensor(
                    out=o2[:], in0=psW[:, 0:OUT],
                    in1=recip[:][:, 0:1].to_broadcast([128, OUT]), op=OP.mult)
                nc.vector.tensor_tensor(out=o2[:], in0=o2[:], in1=b2_t[:], op=OP.add)
                eo = sb.tile([128, OUT], dt.float32, tag="eo")
                ssum = sb.tile([128, 1], dt.float32, tag="ssum")
                nc.scalar.activation(out=eo[:], in_=o2[:], func=AF.Exp, accum_out=ssum[:])
                lns = sb.tile([128, 1], dt.float32, tag="lns")
                nc.scalar.activation(out=lns[:], in_=ssum[:], func=AF.Ln)
                ls = sb.tile([128, OUT], dt.float32, tag="ls")
                nc.vector.tensor_scalar(out=ls[:], in0=o2[:], scalar1=lns[:, 0:1],
                                        scalar2=None, op0=OP.subtract)
                rows = 128 if w < NWIN - 1 else LAST_ROWS
                nc.sync.dma_start(out=out[w * 128:w * 128 + rows, :], in_=ls[:rows, :])

            edge_layer(h2full[:, :], OUT, 1,
                       lambda w: (h2own[w * 128:min((w + 1) * 128, NLOC), 65:66],
                                  128 if w < NWIN - 1 else LAST_ROWS), flush2)

    nc.finalize()
    return nc


_CACHE = {}


def _prepare(x, edge_index, W1, att_src1, att_dst1, bias1, W2, att_src2,
             att_dst2, bias2):
    import ml_dtypes
    import concourse.bass  # noqa: F401  (ensures env boot)

    bf16 = ml_dtypes.bfloat16
    x = np.asarray(x, np.float32)
    xT = np.ascontiguousarray(x.T).astype(bf16)                 # [128, N]
    W1ext, W2ext = _pack_weights(W1, att_src1, att_dst1, W2, att_src2, att_dst2)
    b1_2d = np.broadcast_to(np.asarray(bias1, np.float32)[None, :], (128, C1)).copy()
    b2_2d = np.broadcast_to(np.asarray(bias2, np.float32)[None, :], (128, OUT)).copy()
    iota2d = np.broadcast_to(np.arange(128, dtype=np.float32)[None, :],
                             (128, 128)).astype(bf16).copy()
    ident = np.eye(128, dtype=np.float32).astype(bf16)

    srcsT, dstwT, ngroups, G = _preprocess(np.asarray(edge_index))
    xownT_all = np.zeros((NC, IN, NWIN * 128), np.float32)
    for k in range(NC):
        xownT_all[k, :, :NLOC] = np.asarray(x.T)[:, k * NLOC:(k + 1) * NLOC]
    xownT_all = xownT_all.astype(bf16)

    key = ("nc", G, tuple(ngroups))
    if key not in _CACHE:
        _CACHE[key] = _build_nc(ngroups, G)
    nc = _CACHE[key]

    in_maps = []
    for k in range(NC):
        in_maps.append({
            "xT": xT, "W1e": W1ext, "W2e": W2ext, "iota2d": iota2d,
            "ident": ident, "b1_2d": b1_2d, "b2_2d": b2_2d,
            "srcsT": srcsT[k], "dstwT": dstwT[k], "xownT": xownT_all[k],
        })
    return nc, in_maps


def kernel(x, edge_index, W1, att_src1, att_dst1, bias1, W2, att_src2, att_dst2, bias2):
    from concourse.bass_utils import run_bass_kernel_spmd
    nc, in_maps = _prepare(x, edge_index, W1, att_src1, att_dst1, bias1,
                           W2, att_src2, att_dst2, bias2)
    res = run_bass_kernel_spmd(nc, in_maps, list(range(NC)))
    return np.concatenate([res.results[k]["out"] for k in range(NC)], axis=0)



# revision 15
# speedup vs baseline: 1.0530x; 1.0530x over previous
"""Two-layer GAT on 8 Trainium2 NeuronCores — dst-per-partition layout.

Each destination node owns one SBUF partition; its incoming edges lie along
the free axis. Nodes are degree-sorted on the host and packed into 128-node
windows so every window has near-uniform degree (1.3% pad). One multi-row
indirect DMA per window gathers all source rows (z|a_src), per-edge scores
are vector ops against the per-partition a_dst, and the softmax-weighted
aggregation is a strided free-axis reduce — no one-hot matmuls, no PE
transposes in the edge path, ~20 instructions per window instead of ~120.
Pad edge slots point at a sentinel table row with a_src=-100 (exp -> 0) and
z=0, so they contribute exactly nothing.
"""

import numpy as np

# ---- problem constants (hardcoded per harness contract) ----
N = 100000
E = 1600000
IN = 128
HID = 16
HEADS = 8
OUT = 64
NEG = 0.2
NC = 8
NWIN = 98
NLOC2 = NWIN * 128          # 12544 slots per core
SLOT = NC * NLOC2           # 100352 global slots
ROW1 = HEADS * HID + 2 * HEADS  # 144 = z(128) | a_src(8) | a_dst(8)
ROW1G = HEADS * HID + HEADS     # 136 = gathered row: z | a_src
C1 = HEADS * HID            # 128
ROW2 = OUT + 2              # 66 = h2(64) | a_src2(1) | a_dst2(1)
H2ROWS = NLOC2 + 1          # h2own rows incl sentinel pad row
PHA_B = 3                   # phase-A node tiles per psum bank (3*144=432<=512)


def _preprocess(edge_index):
    """Degree-sorted slot assignment + per-core gather index tables."""
    src = np.concatenate([np.asarray(edge_index[0]), np.arange(N)]).astype(np.int64)
    dst = np.concatenate([np.asarray(edge_index[1]), np.arange(N)]).astype(np.int64)
    E2 = src.shape[0]
    deg = np.bincount(dst, minlength=N)
    order = np.argsort(-deg, kind="stable")
    sl = np.empty(N, dtype=np.int64)
    sl[order] = np.arange(N)

    dsorted = deg[order]
    Db = np.ones(NWIN, dtype=np.int64)
    for b in range(NWIN):
        if b * 1024 < N:
            Db[b] = max(1, int(dsorted[b * 1024]))
    offs = np.concatenate([[0], np.cumsum(Db)]).astype(np.int64)
    SD = int(Db.sum())

    def slot_decomp(s):
        b = s // 1024
        r = (s // 128) % 8
        d = s % 128
        core = np.where(b % 2 == 0, r, 7 - r)
        return core, b, d

    # edge fill: group edges by dst slot, j = running index within dst
    sd = sl[dst]
    o = np.argsort(sd, kind="stable")
    sd_s, src_s = sd[o], src[o]
    start = np.searchsorted(sd_s, np.arange(SLOT))
    j = np.arange(E2) - start[sd_s]
    core_e, pos_e, d_e = slot_decomp(sd_s)
    col = offs[pos_e] + j
    assert (j < Db[pos_e]).all()

    idxT = np.full((NC, 128, SD), N, dtype=np.int32)          # pad -> sentinel
    l2idxT = np.full((NC, 128, SD), NLOC2, dtype=np.int32)    # pad -> core0 row
    flat = core_e * (128 * SD) + d_e * SD + col
    idxT.reshape(-1)[flat] = src_s.astype(np.int32)
    ts = sl[src_s]
    core_s, pos_s, d_s = slot_decomp(ts)
    l2 = core_s * H2ROWS + pos_s * 128 + d_s
    l2idxT.reshape(-1)[flat] = l2.astype(np.int32)

    # slot -> node mapping (N = no node / pad)
    s_all = np.arange(SLOT)
    core_a, pos_a, d_a = slot_decomp(s_all)
    loc_a = pos_a * 128 + d_a
    node_of = np.concatenate([order, np.full(SLOT - N, N, dtype=np.int64)])
    node_slot = np.empty(SLOT, dtype=np.int64)
    # slot s holds node node_of[s]
    perm = np.full((NC, NLOC2), N, dtype=np.int64)
    perm[core_a, loc_a] = node_of[s_all]
    return idxT, l2idxT, Db.tolist(), SD, perm


def _pack_weights(W1, att_src1, att_dst1, W2, att_src2, att_dst2):
    import ml_dtypes
    W1 = np.asarray(W1, np.float32)
    W2 = np.asarray(W2, np.float32)
    A1s = np.zeros((C1, HEADS), np.float32)
    A1d = np.zeros((C1, HEADS), np.float32)
    for h in range(HEADS):
        A1s[h * HID:(h + 1) * HID, h] = np.asarray(att_src1, np.float32)[h]
        A1d[h * HID:(h + 1) * HID, h] = np.asarray(att_dst1, np.float32)[h]
    W1ext = np.concatenate([W1, W1 @ A1s, W1 @ A1d], axis=1)   # [128, 144]
    W2ext = np.concatenate(
        [W2, W2 @ np.asarray(att_src2, np.float32).T,
         W2 @ np.asarray(att_dst2, np.float32).T], axis=1)     # [128, 66]
    return (np.ascontiguousarray(W1ext).astype(ml_dtypes.bfloat16),
            np.ascontiguousarray(W2ext).astype(ml_dtypes.bfloat16))


def _build_nc(Dlist, SD):
    import concourse.bass as bass
    import concourse.bacc as bacc
    import concourse.mybir as mybir
    import concourse.tile as tile

    dt = mybir.dt
    AF = mybir.ActivationFunctionType
    OP = mybir.AluOpType
    nc = bacc.Bacc("TRN2", target_bir_lowering=False, debug=False, num_devices=NC)

    xT = nc.dram_tensor("xT", [IN, N], dt.bfloat16, kind="ExternalInput")
    W1e = nc.dram_tensor("W1e", [IN, ROW1], dt.bfloat16, kind="ExternalInput")
    W2e = nc.dram_tensor("W2e", [C1, ROW2], dt.bfloat16, kind="ExternalInput")
    ident = nc.dram_tensor("ident", [128, 128], dt.bfloat16, kind="ExternalInput")
    b1_2d = nc.dram_tensor("b1_2d", [128, C1], dt.float32, kind="ExternalInput")
    b2_2d = nc.dram_tensor("b2_2d", [128, OUT], dt.float32, kind="ExternalInput")
    pads = nc.dram_tensor("pads", [1, ROW1G + ROW2], dt.bfloat16,
                          kind="ExternalInput")
    idxD = nc.dram_tensor("idxD", [128, SD], dt.int32, kind="ExternalInput")
    l2idxD = nc.dram_tensor("l2idxD", [128, SD], dt.int32, kind="ExternalInput")
    xownT = nc.dram_tensor("xownT", [IN, NLOC2], dt.bfloat16, kind="ExternalInput")
    out = nc.dram_tensor("out", [NLOC2, OUT], dt.float32, kind="ExternalOutput")

    import os
    dbg = os.environ.get("GAT_DEBUG_H2") == "1"
    h2dbg = (nc.dram_tensor("h2dbg", [128, NWIN * ROW2], dt.float32,
                            kind="ExternalOutput") if dbg else None)
    h2fdbg = (nc.dram_tensor("h2fdbg", [NC * H2ROWS, ROW2], dt.bfloat16,
                             kind="ExternalOutput") if dbg else None)
    e2dbg = (nc.dram_tensor("e2dbg", [128, 2 * NWIN], dt.float32,
                            kind="ExternalOutput") if dbg else None)
    hext1 = nc.dram_tensor("hext1", [N + 1, ROW1G], dt.bfloat16)
    h2own = nc.dram_tensor("h2own", [H2ROWS, ROW2], dt.bfloat16)
    h2full = nc.dram_tensor("h2full", [NC * H2ROWS, ROW2], dt.bfloat16,
                            addr_space="Shared")

    offs = np.concatenate([[0], np.cumsum(Dlist)]).astype(int)

    with tile.TileContext(nc) as tc:
        with tc.tile_pool(name="const", bufs=1) as cb:
            w1e_t = cb.tile([IN, ROW1], dt.bfloat16)
            nc.sync.dma_start(out=w1e_t[:], in_=W1e[:, :])
            w2e_t = cb.tile([C1, ROW2], dt.bfloat16)
            nc.sync.dma_start(out=w2e_t[:], in_=W2e[:, :])
            ident_t = cb.tile([128, 128], dt.bfloat16)
            nc.sync.dma_start(out=ident_t[:], in_=ident[:, :])
            b1_t = cb.tile([128, C1], dt.float32)
            nc.sync.dma_start(out=b1_t[:], in_=b1_2d[:, :])
            b2_t = cb.tile([128, OUT], dt.float32)
            nc.sync.dma_start(out=b2_t[:], in_=b2_2d[:, :])
            pads_t = cb.tile([1, ROW1G + ROW2], dt.bfloat16)
            nc.sync.dma_start(out=pads_t[:], in_=pads[:, :])
            idx_t = cb.tile([128, SD], dt.int32)
            nc.sync.dma_start(out=idx_t[:], in_=idxD[:, :])
            l2idx_t = cb.tile([128, SD], dt.int32)
            nc.sync.dma_start(out=l2idx_t[:], in_=l2idxD[:, :])
            # resident accumulators
            adw_t = cb.tile([128, NWIN * HEADS], dt.bfloat16)
            h2_all = cb.tile([128, NWIN * ROW2], dt.bfloat16)
            adw2f = cb.tile([128, NWIN], dt.float32)
            o2_all = cb.tile([128, NWIN * OUT], dt.float32)
            ssum_all = cb.tile([128, NWIN], dt.float32)
            tc.strict_bb_all_engine_barrier()

            # sentinel rows: hext1[N] = (z=0 | a_src=-100); h2own[NLOC2] likewise
            nc.sync.dma_start(out=hext1[N:N + 1, :], in_=pads_t[:, 0:ROW1G])
            nc.sync.dma_start(out=h2own[NLOC2:NLOC2 + 1, :],
                              in_=pads_t[:, ROW1G:ROW1G + ROW2])

            # ---------------- Phase A: hext1 = x @ [W1|W1@A1s] -------------
            with (
                tc.tile_pool(name="pha_sb", bufs=3) as sa,
                tc.tile_pool(name="pha_ps", bufs=2, space="PSUM") as pa,
            ):
                ntile = (N + 127) // 128  # 782, last has 32 rows
                t = 0
                while t < ntile:
                    nb = min(PHA_B, ntile - t)
                    r0 = t * 128
                    rows = min(nb * 128, N - r0)
                    xt = sa.tile([IN, nb * 128], dt.bfloat16, tag="xt")
                    nc.sync.dma_start(out=xt[:, :rows], in_=xT[:, r0:r0 + rows])
                    psA = pa.tile([128, nb * ROW1G], dt.float32, tag="psA")
                    for b in range(nb):
                        rr = min(128, N - (t + b) * 128)
                        nc.tensor.matmul(
                            out=psA[:rr, b * ROW1G:(b + 1) * ROW1G],
                            lhsT=xt[:, b * 128:b * 128 + rr],
                            rhs=w1e_t[:, 0:ROW1G], start=True, stop=True)
                    zs = sa.tile([128, nb * ROW1G], dt.bfloat16, tag="zs")
                    if rows % 128 == 0:
                        nc.vector.tensor_copy(out=zs[:], in_=psA[:])
                        nc.sync.dma_start(
                            out=hext1[r0:r0 + rows, :].rearrange(
                                "(b p) f -> p b f", p=128),
                            in_=zs[:].rearrange("p (b f) -> p b f", b=nb))
                    else:
                        for b in range(nb):
                            rr = min(128, N - (t + b) * 128)
                            nc.vector.tensor_copy(
                                out=zs[:rr, b * ROW1G:(b + 1) * ROW1G],
                                in_=psA[:rr, b * ROW1G:(b + 1) * ROW1G])
                            nc.sync.dma_start(
                                out=hext1[(t + b) * 128:(t + b) * 128 + rr, :],
                                in_=zs[:rr, b * ROW1G:(b + 1) * ROW1G])
                    t += nb
                # Phase A2: per-slot a_dst table -> resident adw_t
                for w in range(NWIN):
                    xo = sa.tile([IN, 128], dt.bfloat16, tag="xo")
                    nc.sync.dma_start(out=xo[:], in_=xownT[:, w * 128:(w + 1) * 128])
                    psA2 = pa.tile([128, HEADS], dt.float32, tag="psA2")
                    nc.tensor.matmul(out=psA2[:], lhsT=xo[:],
                                     rhs=w1e_t[:, ROW1G:ROW1], start=True, stop=True)
                    nc.scalar.copy(out=adw_t[:, w * HEADS:(w + 1) * HEADS],
                                   in_=psA2[:])
            tc.strict_bb_all_engine_barrier()

            # ---------------- Layer 1 windows ------------------------------
            with (
                tc.tile_pool(name="l1_hx", bufs=3) as hb,
                tc.tile_pool(name="l1_sb", bufs=2) as sb,
                tc.tile_pool(name="l1_ps", bufs=2, space="PSUM") as p1,
            ):
                for w in range(NWIN):
                    D = int(Dlist[w])
                    g0 = int(offs[w])
                    hx = hb.tile([128, D * ROW1G], dt.bfloat16, tag="hx")
                    for j in range(D):
                        nc.gpsimd.indirect_dma_start(
                            out=hx[:, j * ROW1G:(j + 1) * ROW1G],
                            out_offset=None, in_=hext1[:, :],
                            in_offset=bass.IndirectOffsetOnAxis(
                                ap=idx_t[:, g0 + j:g0 + j + 1], axis=0))
                    hxv = hx[:].rearrange("p (j f) -> p j f", j=D)
                    # e = a_src[src] + a_dst[dst]
                    e = sb.tile([128, D * HEADS], dt.float32, tag="e")
                    ev = e[:].rearrange("p (j h) -> p j h", j=D)
                    nc.vector.tensor_tensor(
                        out=ev, in0=hxv[:, :, C1:C1 + HEADS],
                        in1=adw_t[:, w * HEADS:(w + 1) * HEADS][:, None, :]
                        .to_broadcast([128, D, HEADS]), op=OP.add)
                    sc = sb.tile([128, D * HEADS], dt.float32, tag="sc")
                    nc.vector.tensor_scalar(out=sc[:], in0=e[:], scalar1=NEG,
                                            scalar2=None, op0=OP.mult)
                    w8f = sb.tile([128, D * HEADS], dt.float32, tag="w8f")
                    nc.vector.tensor_tensor(out=w8f[:], in0=e[:], in1=sc[:],
                                            op=OP.max)
                    w8 = sb.tile([128, D * HEADS], dt.bfloat16, tag="w8")
                    nc.scalar.activation(out=w8[:], in_=w8f[:], func=AF.Exp)
                    den = sb.tile([128, HEADS], dt.float32, tag="den")
                    nc.vector.tensor_reduce(
                        out=den[:], in_=w8[:].rearrange("p (j h) -> p h j", j=D),
                        axis=mybir.AxisListType.X, op=OP.add)
                    # weighted features and aggregation
                    whr = sb.tile([128, D * C1], dt.bfloat16, tag="whr")
                    nc.vector.tensor_tensor(
                        out=whr[:].rearrange("p (j h c) -> p j h c", j=D, h=HEADS),
                        in0=hxv[:, :, 0:C1].rearrange("p j (h c) -> p j h c",
                                                      h=HEADS),
                        in1=w8[:].rearrange("p (j h) -> p j h", j=D)[:, :, :, None]
                        .to_broadcast([128, D, HEADS, HID]), op=OP.mult)
                    agg = sb.tile([128, C1], dt.float32, tag="agg")
                    nc.vector.tensor_reduce(
                        out=agg[:], in_=whr[:].rearrange("p (j c) -> p c j", j=D),
                        axis=mybir.AxisListType.X, op=OP.add)
                    recip = sb.tile([128, HEADS], dt.float32, tag="recip")
                    nc.vector.tensor_scalar(out=recip[:], in0=den[:],
                                            scalar1=1e-16, scalar2=None, op0=OP.add)
                    nc.vector.reciprocal(out=recip[:], in_=recip[:])
                    A = sb.tile([128, C1], dt.bfloat16, tag="A")
                    nc.vector.tensor_tensor(
                        out=A[:].rearrange("p (h c) -> p h c", h=HEADS),
                        in0=agg[:].rearrange("p (h c) -> p h c", h=HEADS),
                        in1=recip[:][:, :, None].to_broadcast([128, HEADS, HID]),
                        op=OP.mult)
                    nc.vector.tensor_tensor(out=A[:], in0=A[:], in1=b1_t[:],
                                            op=OP.add)
                    nc.vector.tensor_scalar(out=A[:], in0=A[:], scalar1=0.0,
                                            scalar2=None, op0=OP.max)
                    # layer-2 transform: h2 = A @ W2ext
                    psT = p1.tile([128, 128], dt.bfloat16, tag="psT")
                    nc.tensor.transpose(out=psT[:], in_=A[:], identity=ident_t[:])
                    at = sb.tile([128, 128], dt.bfloat16, tag="at")
                    nc.scalar.copy(out=at[:], in_=psT[:])
                    ps2 = p1.tile([128, ROW2], dt.float32, tag="ps2")
                    nc.tensor.matmul(out=ps2[:], lhsT=at[:], rhs=w2e_t[:],
                                     start=True, stop=True)
                    nc.scalar.copy(out=h2_all[:, w * ROW2:(w + 1) * ROW2],
                                   in_=ps2[:])
                    nc.scalar.copy(out=adw2f[:, w:w + 1],
                                   in_=ps2[:, OUT + 1:OUT + 2])
            # write h2own (one strided DMA) and allgather
            if dbg:
                h2f = cb.tile([128, NWIN * ROW2], dt.float32)
                nc.vector.tensor_copy(out=h2f[:], in_=h2_all[:])
                nc.sync.dma_start(out=h2dbg[:, :], in_=h2f[:])
            nc.sync.dma_start(
                out=h2own[0:NLOC2, :].rearrange("(w d) c -> d w c", d=128),
                in_=h2_all[:].rearrange("d (w c) -> d w c", w=NWIN))
            tc.strict_bb_all_engine_barrier()

            nc.gpsimd.collective_compute(
                "AllGather", mybir.AluOpType.bypass,
                replica_groups=[list(range(NC))],
                ins=[h2own[:, :]], outs=[h2full[:, :]])
            tc.strict_bb_all_engine_barrier()
            if dbg:
                nc.sync.dma_start(out=h2fdbg[:, :], in_=h2full[:, :])
                tc.strict_bb_all_engine_barrier()

            # ---------------- Layer 2 windows ------------------------------
            if dbg:
                e2mx = cb.tile([128, NWIN], dt.float32)
                d2all = cb.tile([128, NWIN], dt.float32)
            with (
                tc.tile_pool(name="l2_hx", bufs=3) as hb2,
                tc.tile_pool(name="l2_sb", bufs=2) as sb2,
            ):
                for w in range(NWIN):
                    D = int(Dlist[w])
                    g0 = int(offs[w])
                    hx2 = hb2.tile([128, D * ROW2], dt.bfloat16, tag="hx2")
                    for j in range(D):
                        nc.gpsimd.indirect_dma_start(
                            out=hx2[:, j * ROW2:(j + 1) * ROW2],
                            out_offset=None, in_=h2full[:, :],
                            in_offset=bass.IndirectOffsetOnAxis(
                                ap=l2idx_t[:, g0 + j:g0 + j + 1], axis=0))
                    hx2v = hx2[:].rearrange("p (j f) -> p j f", j=D)
                    e2 = sb2.tile([128, D], dt.float32, tag="e2")
                    nc.vector.tensor_scalar(
                        out=e2[:], in0=hx2v[:, :, OUT],
                        scalar1=adw2f[:, w:w + 1],
                        scalar2=None, op0=OP.add)
                    sc2 = sb2.tile([128, D], dt.float32, tag="sc2")
                    nc.vector.tensor_scalar(out=sc2[:], in0=e2[:], scalar1=NEG,
                                            scalar2=None, op0=OP.mult)
                    w8f2 = sb2.tile([128, D], dt.float32, tag="w8f2")
                    nc.vector.tensor_tensor(out=w8f2[:], in0=e2[:], in1=sc2[:],
                                            op=OP.max)
                    w82 = sb2.tile([128, D], dt.bfloat16, tag="w82")
                    nc.scalar.activation(out=w82[:], in_=w8f2[:], func=AF.Exp)
                    den2 = sb2.tile([128, 1], dt.float32, tag="den2")
                    nc.vector.tensor_reduce(
                        out=den2[:], in_=w82[:], axis=mybir.AxisListType.X,
                        op=OP.add)
                    if dbg:
                        nc.vector.tensor_reduce(
                            out=e2mx[:, w:w + 1], in_=e2[:],
                            axis=mybir.AxisListType.X, op=OP.max)
                        nc.vector.tensor_copy(out=d2all[:, w:w + 1], in_=den2[:])
                    wh2 = sb2.tile([128, D * OUT], dt.bfloat16, tag="wh2")
                    nc.vector.tensor_tensor(
                        out=wh2[:].rearrange("p (j c) -> p j c", j=D),
                        in0=hx2v[:, :, 0:OUT],
                        in1=w82[:][:, :, None].to_broadcast([128, D, OUT]),
                        op=OP.mult)
                    agg2 = sb2.tile([128, OUT], dt.float32, tag="agg2")
                    nc.vector.tensor_reduce(
                        out=agg2[:], in_=wh2[:].rearrange("p (j c) -> p c j", j=D),
                        axis=mybir.AxisListType.X, op=OP.add)
                    recip2 = sb2.tile([128, 1], dt.float32, tag="recip2")
                    nc.vector.tensor_scalar(out=recip2[:], in0=den2[:],
                                            scalar1=1e-16, scalar2=None,
                                            op0=OP.add)
                    nc.vector.reciprocal(out=recip2[:], in_=recip2[:])
                    o2 = o2_all[:, w * OUT:(w + 1) * OUT]
                    nc.vector.tensor_scalar(out=o2, in0=agg2[:],
                                            scalar1=recip2[:, 0:1], scalar2=None,
                                            op0=OP.mult)
                    nc.vector.tensor_tensor(out=o2, in0=o2, in1=b2_t[:], op=OP.add)
                    eo = sb2.tile([128, OUT], dt.float32, tag="eo")
                    nc.scalar.activation(out=eo[:], in_=o2, func=AF.Exp,
                                         accum_out=ssum_all[:, w:w + 1])
                # log-softmax epilogue (batched)
                lns = cb.tile([128, NWIN], dt.float32)
                nc.scalar.activation(out=lns[:], in_=ssum_all[:], func=AF.Ln)
                nc.vector.tensor_tensor(
                    out=o2_all[:].rearrange("p (w c) -> p w c", w=NWIN),
                    in0=o2_all[:].rearrange("p (w c) -> p w c", w=NWIN),
                    in1=lns[:][:, :, None].to_broadcast([128, NWIN, OUT]),
                    op=OP.subtract)
                nc.sync.dma_start(
                    out=out[:, :].rearrange("(w d) c -> d w c", d=128),
                    in_=o2_all[:].rearrange("d (w c) -> d w c", w=NWIN))
                if dbg:
                    nc.sync.dma_start(out=e2dbg[:, 0:NWIN], in_=e2mx[:])
                    nc.sync.dma_start(out=e2dbg[:, NWIN:2 * NWIN], in_=d2all[:])

    nc.finalize()
    return nc


_CACHE = {}


def _prepare(x, edge_index, W1, att_src1, att_dst1, bias1, W2, att_src2,
             att_dst2, bias2):
    import ml_dtypes
    import concourse.bass  # noqa: F401  (ensures env boot)

    bf16 = ml_dtypes.bfloat16
    x = np.asarray(x, np.float32)
    xT = np.ascontiguousarray(x.T).astype(bf16)                 # [128, N]
    W1ext, W2ext = _pack_weights(W1, att_src1, att_dst1, W2, att_src2, att_dst2)
    b1_2d = np.broadcast_to(np.asarray(bias1, np.float32)[None, :], (128, C1)).copy()
    b2_2d = np.broadcast_to(np.asarray(bias2, np.float32)[None, :], (128, OUT)).copy()
    ident = np.eye(128, dtype=np.float32).astype(bf16)
    pads = np.zeros((1, ROW1G + ROW2), np.float32)
    pads[0, C1:C1 + HEADS] = -100.0          # layer-1 sentinel a_src
    pads[0, ROW1G + OUT] = -100.0            # layer-2 sentinel a_src2
    pads = pads.astype(bf16)

    idxT, l2idxT, Dlist, SD, perm = _preprocess(np.asarray(edge_index))
    xT_ext = np.concatenate([xT, np.zeros((IN, 1), xT.dtype)], axis=1)
    xownT_all = np.zeros((NC, IN, NLOC2), np.float32)
    for k in range(NC):
        xownT_all[k] = xT_ext[:, perm[k]].astype(np.float32)
    xownT_all = xownT_all.astype(bf16)

    key = ("nc", SD, tuple(Dlist))
    if key not in _CACHE:
        _CACHE[key] = _build_nc(Dlist, SD)
    nc = _CACHE[key]

    in_maps = []
    for k in range(NC):
        in_maps.append({
            "xT": xT, "W1e": W1ext, "W2e": W2ext, "ident": ident,
            "b1_2d": b1_2d, "b2_2d": b2_2d, "pads": pads,
            "idxD": idxT[k], "l2idxD": l2idxT[k], "xownT": xownT_all[k],
        })
    return nc, in_maps, perm


def kernel(x, edge_index, W1, att_src1, att_dst1, bias1, W2, att_src2,
           att_dst2, bias2):
    from concourse.bass_utils import run_bass_kernel_spmd
    nc, in_maps, perm = _prepare(x, edge_index, W1, att_src1, att_dst1, bias1,
                                 W2, att_src2, att_dst2, bias2)
    res = run_bass_kernel_spmd(nc, in_maps, list(range(NC)))
    out_full = np.zeros((N, OUT), np.float32)
    for k in range(NC):
        o = np.asarray(res.results[k]["out"])
        m = perm[k] < N
        out_full[perm[k][m]] = o[m]
    return out_full
